# revision 1
# baseline (speedup 1.0000x reference)
"""MoE gate (DeepSeek-V2 style, group-limited greedy top-k) for Trainium2.

Full-input contract: kernel(hidden_states[4,8192,2048] f32, kernel[64,2048] f32)
-> topk_weight [32768, 6] f32.

Strategy: pure data-parallel over 8 NeuronCores (4096 tokens each).
Per core:
  - tokens are remapped so partition p owns a contiguous 32-token DRAM range
    (t = p*32 + m*4 + b), making every DMA descriptor large & contiguous.
  - per 512-token megatile: DMA x -> SBUF [128, 4, 2048]; PE-transpose
    (float32r mode, exact fp32 bits) into PSUM; copy PSUM->SBUF xT
    [128h, 512t] alternating ACT/DVE engines; accumulate logitsT[64, 512]
    over 16 h-chunks with float32r matmuls (W stationary); PE-transpose
    logits back to [128t, 64e]; then a per-128-token top-k pipeline on
    DVE/ACT using the hardware top-8 sort (InstMax):
      softmax denominator cancels in the final normalization, so we only
      need e = exp(logit - max); group-max -> sort -> 3rd value threshold
      -> group mask -> masked e -> top-8 sort -> sum top-6 -> reciprocal
      -> scale.
"""

import sys

if "/opt/trn_rl_repo" not in sys.path:
    sys.path.insert(0, "/opt/trn_rl_repo")

import numpy as np

# Problem constants (hardcoded per contract)
N_CORES = 8
H = 2048
E = 64  # n_routed_experts
G = 8  # n_group
PG = E // G  # experts per group
TG = 3  # topk_group
TK = 6  # top_k
P = 128  # partitions
MEGA = 512  # tokens per megatile
BB = MEGA // P  # 4 token blocks per megatile
KCH = H // P  # 16 contraction chunks


def build_nc(t_core, repeat=1):
    """Build the single-core Bass program for a t_core-token shard.

    repeat>1 re-runs the whole pipeline (timing experiments only).
    """
    from concourse import bacc, mybir, masks
    from concourse.tile import TileContext

    f32 = mybir.dt.float32
    f32r = mybir.dt.float32r
    X = mybir.AxisListType.X
    NM = t_core // MEGA
    assert t_core % MEGA == 0

    nc = bacc.Bacc()
    x = nc.declare_dram_parameter("x", [t_core, H], f32, isOutput=False)
    w = nc.declare_dram_parameter("w", [E, H], f32, isOutput=False)
    out = nc.declare_dram_parameter("out", [t_core, TK], f32, isOutput=True)

    with TileContext(nc) as tc:
        with (
            tc.tile_pool(name="const", bufs=1) as cpool,
            tc.tile_pool(name="xin", bufs=6) as xpool,
            tc.tile_pool(name="xhi", bufs=2) as xhipool,
            tc.tile_pool(name="xlo", bufs=2) as xlopool,
            tc.tile_pool(name="lts", bufs=2) as ltspool,
            tc.tile_pool(name="small", bufs=2) as spool,
            tc.tile_pool(name="outp", bufs=2) as opool,
            tc.tile_pool(name="ps_t", bufs=5, space="PSUM") as pst,
            tc.tile_pool(name="ps_mm", bufs=2, space="PSUM") as psmm,
            tc.tile_pool(name="ps_lg", bufs=1, space="PSUM") as pslg,
        ):
            identf = cpool.tile([P, P], f32)
            masks.make_identity(nc, identf[:])
            idf = identf[:]

            w_sb = cpool.tile([E, H], f32)
            w_hi = cpool.tile([P, KCH, E], f32r)
            w_lo = cpool.tile([P, KCH, E], f32r)

            def warm_pe(n=24):
                # Dummy identity transposes fill the otherwise-idle DMA head
                # and burn through the PE p-state ramp (P3/HAM warmup), so
                # real transposes start at full clock.
                pwm = pslg.tile([P, P], f32, tag="lg")
                for _ in range(n):
                    nc.tensor.transpose(pwm[:], idf, idf)

            def setup_w():
                # W: load + transpose once -> w_hi/w_lo [128h, k, 64e] f32r
                # (hi/lo split so that 3 f32r matmuls reach fp32 accuracy).
                # Issued after megatile 0's loads so it doesn't gate the head;
                # chunked so the first W transposes start early.
                nc.scalar.dma_start(out=w_sb[:], in_=w[:])
                for k in range(KCH):
                    pw = psmm.tile([P, E], f32, tag="lt")
                    nc.tensor.transpose(
                        pw[:, 0:E],
                        w_sb[:, k * P : (k + 1) * P],
                        idf[0:E, 0:E],
                    )
                    nc.vector.tensor_copy(w_hi[:, k, :], pw[:, 0:E])
                    nc.vector.tensor_tensor(
                        w_lo[:, k, :], pw[:, 0:E], w_hi[:, k, :],
                        mybir.AluOpType.subtract,
                    )

            xr = x[:].rearrange("(p m b) h -> p m b h", p=P, m=NM, b=BB)
            our = out[:].rearrange("(p m b) k -> p m b k", p=P, m=NM, b=BB)

            def load_and_transpose(m, hsplit=False):
                # Loads alternate the two HWDGE rings (SP + ACT). Steady
                # state: one load per token-quarter. Megatile 0 (hsplit):
                # split along H instead, so transpose chunk k waits only on
                # h-quarter k//4 and the pipeline fills ~3us earlier.
                xq = []
                HQ = H // BB
                for c in range(BB):
                    eng = nc.sync if c % 2 == 0 else nc.scalar
                    if hsplit:
                        t = xpool.tile([P, BB, HQ], f32, tag="xin")
                        eng.dma_start(
                            out=t[:], in_=xr[:, m, :, c * HQ : (c + 1) * HQ]
                        )
                    else:
                        t = xpool.tile([P, H], f32, tag="xin")
                        eng.dma_start(out=t[:], in_=xr[:, m, c, :])
                    xq.append(t)

                def src(k, b):
                    if hsplit:
                        kq = HQ // P
                        return xq[k // kq][:, b, (k % kq) * P : (k % kq + 1) * P]
                    return xq[b][:, k * P : (k + 1) * P]

                x_hi = xhipool.tile([P, KCH, MEGA], f32r)
                x_lo = xlopool.tile([P, KCH, MEGA], f32r)
                for k in range(KCH):
                    pt = pst.tile([P, MEGA], f32, tag="pt")
                    for b in range(BB):
                        nc.tensor.transpose(
                            pt[:, b * P : (b + 1) * P],
                            src(k, b),
                            idf,
                        )
                    # hi = f32r(x) on ACT (1-input); lo = f32r(x - hi) on DVE
                    nc.scalar.copy(x_hi[:, k, :], pt[:])
                    nc.vector.tensor_tensor(
                        x_lo[:, k, :], pt[:], x_hi[:, k, :],
                        mybir.AluOpType.subtract,
                    )
                return x_hi, x_lo

            def compute(m, x_hi, x_lo, t0=0, width=MEGA):
                nb = width // P  # token blocks in this slice
                b0 = t0 // P
                # logitsT[64, width] += w_hi.x_hi + w_hi.x_lo + w_lo.x_hi
                lt = psmm.tile([E, width], f32, tag="lt")
                n_acc = 3 * KCH
                i_acc = 0
                for k in range(KCH):
                    for wt_k, xt_k in (
                        (w_hi, x_hi),
                        (w_hi, x_lo),
                        (w_lo, x_hi),
                    ):
                        nc.tensor.matmul(
                            lt[:],
                            wt_k[:, k, :],
                            xt_k[:, k, t0 : t0 + width],
                            start=(i_acc == 0),
                            stop=(i_acc == n_acc - 1),
                        )
                        i_acc += 1
                lts = ltspool.tile([E, width], f32, tag="lts")
                nc.scalar.copy(lts[:], lt[:])

                # transpose logits back -> [128t, 64e] blocks in PSUM (fp32)
                lg = pslg.tile([P, nb * E], f32, tag="lg")
                for b in range(nb):
                    nc.tensor.transpose(
                        lg[:, b * E : (b + 1) * E],
                        lts[:, b * P : (b + 1) * P],
                        idf[0:E, 0:E],
                    )

                # --- top-k pipeline, all nb token-blocks fused per op ---
                BB = nb
                lg3 = lg[:].rearrange("p (b e) -> p b e", b=BB)  # [128,nb,64]
                # e = exp(logit - max): keeps ACT exp args in [-24, 0] where
                # the table is ~4x more accurate (fewer selection-flip risks
                # near group-boundary ties). Per-block bias via DVE subtract.
                nmax = spool.tile([P, BB], f32, tag="nmax")
                nc.vector.tensor_reduce(
                    nmax[:], lg3, axis=X, op=mybir.AluOpType.max, negate=True
                )
                lsub = spool.tile([P, BB, E], f32, tag="lsub")
                nc.vector.tensor_tensor(
                    lsub[:],
                    lg3,
                    nmax[:].unsqueeze(2).broadcast_to([P, BB, E]),
                    mybir.AluOpType.add,
                )
                e_sb = spool.tile([P, BB, E], f32, tag="esb")
                nc.scalar.activation(
                    e_sb[:], lsub[:], mybir.ActivationFunctionType.Exp
                )
                e4 = e_sb[:].rearrange("p b (g j) -> p b g j", g=G)
                gmax = spool.tile([P, BB, G], f32, tag="gmax")
                nc.vector.tensor_reduce(
                    gmax[:], e4, axis=X, op=mybir.AluOpType.max
                )
                gsort = spool.tile([P, BB, 8], f32, tag="gsort")
                for b in range(BB):
                    nc.vector.max(gsort[:, b, :], gmax[:, b, :])
                gmask = spool.tile([P, BB, G], f32, tag="gmask")
                nc.vector.tensor_tensor(
                    gmask[:],
                    gmax[:],
                    gsort[:, :, TG - 1 : TG].broadcast_to([P, BB, G]),
                    mybir.AluOpType.is_ge,
                )
                me = spool.tile([P, BB, E], f32, tag="me")
                nc.vector.tensor_tensor(
                    me[:].rearrange("p b (g j) -> p b g j", g=G),
                    e4,
                    gmask[:].unsqueeze(3).broadcast_to([P, BB, G, PG]),
                    mybir.AluOpType.mult,
                )
                t8 = spool.tile([P, BB, 8], f32, tag="t8")
                for b in range(BB):
                    nc.vector.max(t8[:, b, :], me[:, b, :])
                ssum = spool.tile([P, BB], f32, tag="ssum")
                nc.vector.tensor_reduce(
                    ssum[:], t8[:, :, 0:TK], axis=X, op=mybir.AluOpType.add
                )
                rec = spool.tile([P, BB], f32, tag="rec")
                nc.vector.reciprocal(rec[:], ssum[:])
                ow = opool.tile([P, BB, TK], f32, tag="ow")
                nc.vector.tensor_tensor(
                    ow[:],
                    t8[:, :, 0:TK],
                    rec[:].unsqueeze(2).broadcast_to([P, BB, TK]),
                    mybir.AluOpType.mult,
                )
                nc.sync.dma_start(out=our[:, m, b0 : b0 + nb], in_=ow[:])

            # two-stage software pipeline: transposes/copies of megatile m
            # are issued alongside the matmuls/topk of megatile m-1 so the
            # PE never waits on PSUM->SBUF copies of the tile it multiplies.
            prev = None
            w_done = False
            warm_pe()
            for _r in range(repeat):
                for m in range(NM):
                    cur = (m, *load_and_transpose(m))
                    if not w_done:
                        setup_w()
                        w_done = True
                    if prev is not None:
                        compute(*prev)
                    prev = cur
            if prev is not None:
                # split the final megatile so its top-k overlaps the second
                # half-chain instead of serializing after the last matmul
                m_l, xh_l, xl_l = prev
                compute(m_l, xh_l, xl_l, 0, MEGA // 2)
                compute(m_l, xh_l, xl_l, MEGA // 2, MEGA // 2)

    nc.compile()
    return nc


_NC_CACHE = {}


def _get_nc(t_core):
    if t_core not in _NC_CACHE:
        _NC_CACHE[t_core] = build_nc(t_core)
    return _NC_CACHE[t_core]


def run_sharded(flat_x, w, trace=False, **kw):
    """flat_x: [T, H] f32. Returns ([T, 6] f32, BassKernelResults)."""
    from concourse.bass_utils import run_bass_kernel_spmd

    T = flat_x.shape[0]
    tc = T // N_CORES
    nc = _get_nc(tc)
    in_maps = [
        {"x": np.ascontiguousarray(flat_x[i * tc : (i + 1) * tc]), "w": w}
        for i in range(N_CORES)
    ]
    res = run_bass_kernel_spmd(nc, in_maps, list(range(N_CORES)), trace=trace, **kw)
    outs = [np.asarray(res.results[i]["out"]) for i in range(N_CORES)]
    return np.concatenate(outs, axis=0), res


def kernel(hidden_states, kernel):
    hs = np.asarray(hidden_states, dtype=np.float32)
    w = np.ascontiguousarray(np.asarray(kernel, dtype=np.float32))
    B, S, Hh = hs.shape
    flat = np.ascontiguousarray(hs.reshape(B * S, Hh))
    out, _ = run_sharded(flat, w)
    return out

